# revision 9
# baseline (speedup 1.0000x reference)
"""3x3 SAME conv (B=32, Cin=128, H=W=64, Cout=256) + bias + relu on 8 trn2 cores.

Strategy: data-parallel over batch (4 images per core, no collectives),
with Winograd F(2,3) along W to cut PE work 1.5x vs direct conv.

The host transforms the input along W (B^T over stride-2 windows of the
zero-padded rows) into 4 t-planes X~t [128cin, 66rows, 32tiles] bf16 per
image, and the weights along kw (G) into W~[chunk, t, kh] [128cin,128cout]
bf16. On device, each iteration (img, chunk, 16-row double-rowgroup) runs
12 matmuls (4 t-planes x 3 kh taps, N=512 moving cols) accumulating 4
PSUM banks Y~t; the Winograd inverse (y_even = Y0+Y1+Y2, y_odd =
Y1-Y2-Y3) runs as 4 fp32 tensor_tensor ops on the vector engine, then
the scalar engine fuses bias+relu+bf16-cast out of SBUF. Even/odd pixel
planes are stored planar to DRAM; the host interleaves and upcasts.

PE work: 32 iters x 12 MM x ~216ns = ~83us; DVE inverse ~84us runs one
iteration behind the PE, psums bufs=8 double-buffers the 4-bank groups.
Startup mirrors the direct-conv baseline: need-ordered sync HWDGE ring
(chunk-0 weights, image-0 row bands, images 1-3), light scalar ring
(bias, chunk-1 weights, stores), and warmup matmuls on a memset tile to
carry the PE clock-gate busy window into the data-ready gate.
"""

from contextlib import ExitStack

import ml_dtypes
import numpy as np

import concourse.bass as bass
import concourse.tile as tile
from concourse import bacc, mybir
from concourse.bass_utils import run_bass_kernel_spmd

N_CORES = 8
B, C_IN, H, W = 32, 128, 64, 64
C_OUT, K = 256, 3
B_LOC = B // N_CORES          # images per core
N_CHUNK = C_OUT // 128        # cout chunks of 128
NT = W // 2                   # Winograd F(2,3) tiles along W
T = 4                         # t-planes
ROWS_PER_IT = 16              # output rows per iteration (N=16*32=512)
N_RG = H // ROWS_PER_IT       # row groups per (image, chunk)
HP = H + 2                    # padded rows

_COMPILED = None


def _build():
    nc = bacc.Bacc("TRN2", target_bir_lowering=False, debug=False,
                   num_devices=N_CORES)

    # X~t planes, indexed [b*T + t] -> [128cin, 66, 32] bf16
    xt = nc.dram_tensor("xt", [B_LOC * T, C_IN, HP, NT], mybir.dt.bfloat16,
                        kind="ExternalInput").ap()
    # W~ chunk-major: wt[c, cin, t*3+kh, m] -> cout m of chunk c
    wt = nc.dram_tensor("wt", [N_CHUNK, C_IN, T * K, 128], mybir.dt.bfloat16,
                        kind="ExternalInput").ap()
    bias2 = nc.dram_tensor("bias2", [128, N_CHUNK], mybir.dt.float32,
                           kind="ExternalInput").ap()
    # planar output: idx = ((b*N_CHUNK + c)*N_RG + d)*2 + parity
    out = nc.dram_tensor("out", [B_LOC * N_CHUNK * N_RG * 2, 128,
                                 ROWS_PER_IT * NT], mybir.dt.bfloat16,
                         kind="ExternalOutput").ap()

    with tile.TileContext(nc) as tc, ExitStack() as ctx:
        consts = ctx.enter_context(tc.tile_pool(name="consts", bufs=1))
        pads = ctx.enter_context(tc.tile_pool(name="pads", bufs=1))
        ys = ctx.enter_context(tc.tile_pool(name="ys", bufs=4))
        outs = ctx.enter_context(tc.tile_pool(name="outs", bufs=6))
        # 4 banks per iteration (4 Y~t planes), double-buffered = all 8 banks
        psums = ctx.enter_context(tc.tile_pool(name="psums", bufs=2,
                                               space="PSUM"))

        w_r = consts.tile([128, N_CHUNK, T * K, 128], mybir.dt.bfloat16,
                          tag="w_r")
        b_sb = consts.tile([128, N_CHUNK], mybir.dt.float32, tag="b_sb")
        nc.sync.dma_start(out=w_r[:, 0], in_=wt[0])
        nc.scalar.dma_start(out=b_sb[:], in_=bias2[:])

        # Warmup matmuls bridge PE dispatch-ready to data-ready so the HAM
        # clock-gate busy window runs into the real stream. They target the
        # first iteration's PSUM tile (overwritten by its start=True matmul)
        # to keep all 8 banks for the double-buffered Y~t groups.
        warm = consts.tile([128, 512], mybir.dt.bfloat16, tag="warm")
        nc.vector.memset(warm[:], 0.0)

        # X~ SBUF tiles: per image [128, T, 66, 32]
        ximgs = [pads.tile([128, T, HP, NT], mybir.dt.bfloat16,
                           name=f"ximg{i}", tag=f"ximg{i}")
                 for i in range(B_LOC)]

        # Need-ordered data ring: image 0 in row bands across all t-planes
        # (iteration d reads rows 16d..16d+17), then images 1-3 whole.
        nc.scalar.dma_start(out=w_r[:, 1], in_=wt[1])
        bounds = [0, 18, 34, 50, HP]
        for s in range(len(bounds) - 1):
            lo, hi = bounds[s], bounds[s + 1]
            for t in range(T):
                nc.sync.dma_start(out=ximgs[0][:, t, lo:hi, :],
                                  in_=xt[t, :, lo:hi, :])
        for b in range(1, B_LOC):
            for t in range(T):
                nc.sync.dma_start(out=ximgs[b][:, t], in_=xt[b * T + t])

        # Per iteration: 12 matmuls fill 4 PSUM banks Y~0..3; the inverse is
        # spread over three engines (DVE TT may read only ONE PSUM input, and
        # gpsimd cannot touch PSUM at all):
        #   scalar: c1=bf16(Y1), c2=bf16(Y2)   (PSUM -> SBUF stages)
        #   gpsimd: vb=c1-c2,  o_od=relu(y_od+bias)  (SBUF only)
        #   DVE:    va=c1+Y0, y_ev=va+c2, y_od=vb-Y3
        #   scalar: o_ev=relu(y_ev+bias)
        # The relu+store epilogue of iteration i is EMITTED after iteration
        # i+1's matmuls so no engine FIFO head-blocks on a cross-engine dep.
        pend = None

        def flush_epilogue(last=False):
            nonlocal pend
            if pend is None:
                return
            y_ev, y_od, cc, oidx = pend
            o_ev = outs.tile([128, ROWS_PER_IT * NT], mybir.dt.bfloat16,
                             name="o_ev", tag="o_ev")
            o_od = outs.tile([128, ROWS_PER_IT * NT], mybir.dt.bfloat16,
                             name="o_od", tag="o_od")
            nc.scalar.activation(o_ev[:], y_ev[:],
                                 mybir.ActivationFunctionType.Relu,
                                 bias=b_sb[:, cc:cc + 1], scale=1.0)
            nc.gpsimd.tensor_scalar(o_od[:], y_od[:], b_sb[:, cc:cc + 1],
                                    0.0, mybir.AluOpType.add,
                                    mybir.AluOpType.max)
            ring = nc.sync if last else nc.scalar
            ring.dma_start(out=out[oidx], in_=o_ev[:])
            nc.scalar.dma_start(out=out[oidx + 1], in_=o_od[:])
            pend = None

        for b in range(B_LOC):
            ximg = ximgs[b]
            for c in range(N_CHUNK):
                for d in range(N_RG):
                    y0r = d * ROWS_PER_IT
                    yt = [psums.tile([128, ROWS_PER_IT * NT],
                                     mybir.dt.float32, name=f"yt{t}",
                                     tag=f"yt{t}")
                          for t in range(T)]
                    if b == 0 and c == 0 and d == 0:
                        for i in range(13):
                            nc.tensor.matmul(yt[0][:, 0:256], warm[:, 0:128],
                                             warm[:, 0:256], start=True,
                                             stop=True)
                    for t in range(T):
                        for kh in range(K):
                            rhs = ximg[:, t, y0r + kh:y0r + kh + ROWS_PER_IT,
                                       :]
                            nc.tensor.matmul(yt[t][:],
                                             w_r[:, c, t * K + kh, :],
                                             rhs,
                                             start=(kh == 0),
                                             stop=(kh == K - 1))
                    flush_epilogue()
                    c1 = ys.tile([128, ROWS_PER_IT * NT], mybir.dt.bfloat16,
                                 tag="c1")
                    c2 = ys.tile([128, ROWS_PER_IT * NT], mybir.dt.bfloat16,
                                 tag="c2")
                    va = ys.tile([128, ROWS_PER_IT * NT], mybir.dt.float32,
                                 tag="va")
                    vb = ys.tile([128, ROWS_PER_IT * NT], mybir.dt.float32,
                                 tag="vb")
                    y_ev = ys.tile([128, ROWS_PER_IT * NT], mybir.dt.float32,
                                   tag="y_ev")
                    y_od = ys.tile([128, ROWS_PER_IT * NT], mybir.dt.float32,
                                   tag="y_od")
                    nc.scalar.copy(c1[:], yt[1][:])
                    nc.scalar.copy(c2[:], yt[2][:])
                    nc.gpsimd.tensor_tensor(vb[:], c1[:], c2[:],
                                            mybir.AluOpType.subtract)
                    nc.vector.tensor_tensor(va[:], c1[:], yt[0][:],
                                            mybir.AluOpType.add)
                    nc.vector.tensor_tensor(y_ev[:], va[:], c2[:],
                                            mybir.AluOpType.add)
                    nc.vector.tensor_tensor(y_od[:], vb[:], yt[3][:],
                                            mybir.AluOpType.subtract)
                    oidx = ((b * N_CHUNK + c) * N_RG + d) * 2
                    pend = (y_ev, y_od, c, oidx)
        flush_epilogue(last=True)

    nc.compile()
    return nc


def _get_compiled():
    global _COMPILED
    if _COMPILED is None:
        _COMPILED = _build()
    return _COMPILED


# F(2,3) transform matrices (host side, fp32 exact)
_BT = np.array([[1, 0, -1, 0], [0, 1, 1, 0], [0, -1, 1, 0], [0, 1, 0, -1]],
               dtype=np.float32)
_G = np.array([[1, 0, 0], [.5, .5, .5], [.5, -.5, .5], [0, 0, 1]],
              dtype=np.float32)


def _run(inp, weight, bias, trace=False):
    inp = np.asarray(inp, dtype=np.float32)
    weight = np.asarray(weight, dtype=np.float32)
    bias = np.asarray(bias, dtype=np.float32)

    # Host: zero-pad, Winograd-transform along W, cast bf16.
    x = np.zeros((B, C_IN, HP, W + 2), dtype=np.float32)
    x[:, :, 1:H + 1, 1:W + 1] = inp
    idx = 2 * np.arange(NT)[:, None] + np.arange(T)[None, :]   # [NT, T]
    xg = x[:, :, :, idx]                                        # [B,C,HP,NT,T]
    xt_full = np.einsum('tk,bchjk->btchj', _BT, xg)             # [B,T,C,HP,NT]
    xt_full = xt_full.astype(ml_dtypes.bfloat16)

    # weight [C_OUT, C_IN*9] -> W~[t,kh,cin,cout] -> [chunk, cin, t*3+kh, m]
    w4 = weight.reshape(C_OUT, C_IN, K, K)
    wtf = np.einsum('tk,ochk->thco', _G, w4)                    # [T,K,C_IN,C_OUT]
    wtd = np.ascontiguousarray(
        wtf.reshape(T * K, C_IN, N_CHUNK, 128).transpose(2, 1, 0, 3)
    ).astype(ml_dtypes.bfloat16)
    bias2 = np.ascontiguousarray(bias.reshape(N_CHUNK, 128).T)

    nc = _get_compiled()
    in_maps = [
        {"xt": np.ascontiguousarray(
            xt_full[i * B_LOC:(i + 1) * B_LOC].reshape(B_LOC * T, C_IN, HP,
                                                       NT)),
         "wt": wtd, "bias2": bias2}
        for i in range(N_CORES)
    ]
    res = run_bass_kernel_spmd(nc, in_maps, list(range(N_CORES)), trace=trace)
    outs = []
    for i in range(N_CORES):
        op = res.results[i]["out"].reshape(B_LOC, N_CHUNK, N_RG, 2, 128,
                                           ROWS_PER_IT, NT)
        # out[b, c*128+m, 16d+h, 2j+par] = op[b, c, d, par, m, h, j]
        full = np.transpose(op, (0, 1, 4, 2, 5, 6, 3)).reshape(
            B_LOC, C_OUT, H, W)
        outs.append(full.astype(np.float32))
    return np.concatenate(outs, axis=0), res


def kernel(inp, weight, bias):
    full, _ = _run(inp, weight, bias, trace=False)
    return full


# revision 13
# speedup vs baseline: 2.6428x; 2.6428x over previous
"""3x3 SAME conv (B=32, Cin=128, H=W=64, Cout=256) + bias + relu on 8 trn2 cores.

Strategy: data-parallel over batch (4 images per core, no collectives),
with Winograd F(2,3) along W to cut PE work 1.5x vs direct conv.

The host transforms the input along W (B^T over stride-2 windows of the
zero-padded rows) into 4 t-planes X~t [128cin, 66rows, 32tiles] bf16 per
image, and the weights along kw (G) into W~[chunk, t, kh] [128cin,128cout]
bf16. On device, each iteration (img, chunk, 16-row double-rowgroup) runs
12 matmuls (4 t-planes x 3 kh taps, N=512 moving cols) accumulating 4
PSUM banks Y~t; the Winograd inverse (y_even = Y0+Y1+Y2, y_odd =
Y1-Y2-Y3) runs as 4 fp32 tensor_tensor ops on the vector engine, then
the scalar engine fuses bias+relu+bf16-cast out of SBUF. Even/odd pixel
planes are stored planar to DRAM; the host interleaves and upcasts.

PE work: 32 iters x 12 MM x ~216ns = ~83us; DVE inverse ~84us runs one
iteration behind the PE, psums bufs=8 double-buffers the 4-bank groups.
Startup mirrors the direct-conv baseline: need-ordered sync HWDGE ring
(chunk-0 weights, image-0 row bands, images 1-3), light scalar ring
(bias, chunk-1 weights, stores), and warmup matmuls on a memset tile to
carry the PE clock-gate busy window into the data-ready gate.
"""

from contextlib import ExitStack

import ml_dtypes
import numpy as np

import concourse.bass as bass
import concourse.tile as tile
from concourse import bacc, mybir
from concourse.bass_utils import run_bass_kernel_spmd

N_CORES = 8
B, C_IN, H, W = 32, 128, 64, 64
C_OUT, K = 256, 3
B_LOC = B // N_CORES          # images per core
N_CHUNK = C_OUT // 128        # cout chunks of 128
NT = W // 2                   # Winograd F(2,3) tiles along W
T = 4                         # t-planes
ROWS_PER_IT = 16              # output rows per iteration (N=16*32=512)
N_RG = H // ROWS_PER_IT       # row groups per (image, chunk)
HP = H + 2                    # padded rows

_COMPILED = None


def _build():
    nc = bacc.Bacc("TRN2", target_bir_lowering=False, debug=False,
                   num_devices=N_CORES)

    # X~t planes, indexed [b*T + t] -> [128cin, 66, 32] bf16
    xt = nc.dram_tensor("xt", [B_LOC * T, C_IN, HP, NT], mybir.dt.bfloat16,
                        kind="ExternalInput").ap()
    # W~ chunk-major: wt[c, cin, t*3+kh, m] -> cout m of chunk c
    wt = nc.dram_tensor("wt", [N_CHUNK, C_IN, T * K, 128], mybir.dt.bfloat16,
                        kind="ExternalInput").ap()
    bias2 = nc.dram_tensor("bias2", [128, N_CHUNK], mybir.dt.float32,
                           kind="ExternalInput").ap()
    # planar output: idx = ((b*N_CHUNK + c)*N_RG + d)*2 + parity
    out = nc.dram_tensor("out", [B_LOC * N_CHUNK * N_RG * 2, 128,
                                 ROWS_PER_IT * NT], mybir.dt.bfloat16,
                         kind="ExternalOutput").ap()

    with tile.TileContext(nc) as tc, ExitStack() as ctx:
        consts = ctx.enter_context(tc.tile_pool(name="consts", bufs=1))
        pads = ctx.enter_context(tc.tile_pool(name="pads", bufs=1))
        ys = ctx.enter_context(tc.tile_pool(name="ys", bufs=4))
        outs = ctx.enter_context(tc.tile_pool(name="outs", bufs=6))
        # 4 banks per iteration (4 Y~t planes), double-buffered = all 8 banks
        psums = ctx.enter_context(tc.tile_pool(name="psums", bufs=2,
                                               space="PSUM"))

        w_r = consts.tile([128, N_CHUNK, T * K, 128], mybir.dt.bfloat16,
                          tag="w_r")
        b_sb = consts.tile([128, N_CHUNK], mybir.dt.float32, tag="b_sb")
        nc.scalar.dma_start(out=b_sb[:], in_=bias2[:])

        # Warmup matmuls bridge PE dispatch-ready to data-ready so the HAM
        # clock-gate busy window runs into the real stream. They target the
        # first iteration's PSUM tile (overwritten by its start=True matmul)
        # to keep all 8 banks for the double-buffered Y~t groups.
        warm = consts.tile([128, 512], mybir.dt.bfloat16, tag="warm")
        nc.vector.memset(warm[:], 0.0)

        # X~ SBUF tiles: per image [128, T, 66, 32]
        ximgs = [pads.tile([128, T, HP, NT], mybir.dt.bfloat16,
                           name=f"ximg{i}", tag=f"ximg{i}")
                 for i in range(B_LOC)]

        # Need-ordered data ring: the first iteration's taps are gated per
        # t-plane (weights for t, then image-0 rows 0..17 of t), so the first
        # matmuls can start before later t-planes land; then the rest of
        # image 0 in row bands, then images 1-3 whole.
        nc.scalar.dma_start(out=w_r[:, 1], in_=wt[1])
        for t in range(T):
            nc.sync.dma_start(out=w_r[:, 0, t * K:(t + 1) * K, :],
                              in_=wt[0, :, t * K:(t + 1) * K, :])
            nc.sync.dma_start(out=ximgs[0][:, t, 0:18, :],
                              in_=xt[t, :, 0:18, :])
        bounds = [18, 34, 50, HP]
        for s in range(len(bounds) - 1):
            lo, hi = bounds[s], bounds[s + 1]
            for t in range(T):
                nc.sync.dma_start(out=ximgs[0][:, t, lo:hi, :],
                                  in_=xt[t, :, lo:hi, :])
        for b in range(1, B_LOC):
            for t in range(T):
                nc.sync.dma_start(out=ximgs[b][:, t], in_=xt[b * T + t])

        # Per iteration: 12 matmuls fill 4 PSUM banks Y~0..3; the inverse
        # splits so DVE never reads two PSUM inputs in one op (HW limit) and
        # the SBUF-only combines run in DVE's 2x bf16 mode:
        #   scalar: c1=bf16(Y1), c2=bf16(Y2)     (PSUM -> SBUF stages)
        #   DVE:    va=c1+Y0 (1x), vb=c1-c2 (2x), y_ev=va+c2 (2x),
        #           y_od=vb-Y3 (1x)              (bf16 intermediates)
        #   scalar: o_ev=relu(y_ev+bias), o_od=relu(y_od+bias)
        # The relu+store epilogue of iteration i is EMITTED after iteration
        # i+1's matmuls so no engine FIFO head-blocks on a cross-engine dep.
        pend = None

        def flush_epilogue(last=False):
            nonlocal pend
            if pend is None:
                return
            y_ev, y_od, cc, oidx = pend
            o_ev = outs.tile([128, ROWS_PER_IT * NT], mybir.dt.bfloat16,
                             name="o_ev", tag="o_ev")
            o_od = outs.tile([128, ROWS_PER_IT * NT], mybir.dt.bfloat16,
                             name="o_od", tag="o_od")
            nc.scalar.activation(o_ev[:], y_ev[:],
                                 mybir.ActivationFunctionType.Relu,
                                 bias=b_sb[:, cc:cc + 1], scale=1.0)
            nc.scalar.activation(o_od[:], y_od[:],
                                 mybir.ActivationFunctionType.Relu,
                                 bias=b_sb[:, cc:cc + 1], scale=1.0)
            ring = nc.sync if last else nc.scalar
            ring.dma_start(out=out[oidx], in_=o_ev[:])
            nc.scalar.dma_start(out=out[oidx + 1], in_=o_od[:])
            pend = None

        for b in range(B_LOC):
            ximg = ximgs[b]
            for c in range(N_CHUNK):
                for d in range(N_RG):
                    y0r = d * ROWS_PER_IT
                    yt = [psums.tile([128, ROWS_PER_IT * NT],
                                     mybir.dt.float32, name=f"yt{t}",
                                     tag=f"yt{t}")
                          for t in range(T)]
                    if b == 0 and c == 0 and d == 0:
                        for i in range(13):
                            nc.tensor.matmul(yt[0][:, 0:256], warm[:, 0:128],
                                             warm[:, 0:256], start=True,
                                             stop=True)
                    for t in range(T):
                        for kh in range(K):
                            rhs = ximg[:, t, y0r + kh:y0r + kh + ROWS_PER_IT,
                                       :]
                            nc.tensor.matmul(yt[t][:],
                                             w_r[:, c, t * K + kh, :],
                                             rhs,
                                             start=(kh == 0),
                                             stop=(kh == K - 1))
                    flush_epilogue()
                    c1 = ys.tile([128, ROWS_PER_IT * NT], mybir.dt.bfloat16,
                                 tag="c1")
                    c2 = ys.tile([128, ROWS_PER_IT * NT], mybir.dt.bfloat16,
                                 tag="c2")
                    va = ys.tile([128, ROWS_PER_IT * NT], mybir.dt.bfloat16,
                                 tag="va")
                    vb = ys.tile([128, ROWS_PER_IT * NT], mybir.dt.bfloat16,
                                 tag="vb")
                    y_ev = ys.tile([128, ROWS_PER_IT * NT], mybir.dt.bfloat16,
                                   tag="y_ev")
                    y_od = ys.tile([128, ROWS_PER_IT * NT], mybir.dt.bfloat16,
                                   tag="y_od")
                    nc.scalar.copy(c1[:], yt[1][:])
                    nc.scalar.copy(c2[:], yt[2][:])
                    nc.vector.tensor_tensor(va[:], c1[:], yt[0][:],
                                            mybir.AluOpType.add)
                    nc.vector.tensor_tensor(vb[:], c1[:], c2[:],
                                            mybir.AluOpType.subtract)
                    nc.vector.tensor_tensor(y_ev[:], va[:], c2[:],
                                            mybir.AluOpType.add)
                    nc.vector.tensor_tensor(y_od[:], vb[:], yt[3][:],
                                            mybir.AluOpType.subtract)
                    oidx = ((b * N_CHUNK + c) * N_RG + d) * 2
                    pend = (y_ev, y_od, c, oidx)
        flush_epilogue(last=True)

    nc.compile()
    return nc


def _get_compiled():
    global _COMPILED
    if _COMPILED is None:
        _COMPILED = _build()
    return _COMPILED


# F(2,3) transform matrices (host side, fp32 exact)
_BT = np.array([[1, 0, -1, 0], [0, 1, 1, 0], [0, -1, 1, 0], [0, 1, 0, -1]],
               dtype=np.float32)
_G = np.array([[1, 0, 0], [.5, .5, .5], [.5, -.5, .5], [0, 0, 1]],
              dtype=np.float32)


def _run(inp, weight, bias, trace=False):
    inp = np.asarray(inp, dtype=np.float32)
    weight = np.asarray(weight, dtype=np.float32)
    bias = np.asarray(bias, dtype=np.float32)

    # Host: zero-pad, Winograd-transform along W, cast bf16.
    x = np.zeros((B, C_IN, HP, W + 2), dtype=np.float32)
    x[:, :, 1:H + 1, 1:W + 1] = inp
    idx = 2 * np.arange(NT)[:, None] + np.arange(T)[None, :]   # [NT, T]
    xg = x[:, :, :, idx]                                        # [B,C,HP,NT,T]
    xt_full = np.einsum('tk,bchjk->btchj', _BT, xg)             # [B,T,C,HP,NT]
    xt_full = xt_full.astype(ml_dtypes.bfloat16)

    # weight [C_OUT, C_IN*9] -> W~[t,kh,cin,cout] -> [chunk, cin, t*3+kh, m]
    w4 = weight.reshape(C_OUT, C_IN, K, K)
    wtf = np.einsum('tk,ochk->thco', _G, w4)                    # [T,K,C_IN,C_OUT]
    wtd = np.ascontiguousarray(
        wtf.reshape(T * K, C_IN, N_CHUNK, 128).transpose(2, 1, 0, 3)
    ).astype(ml_dtypes.bfloat16)
    bias2 = np.ascontiguousarray(bias.reshape(N_CHUNK, 128).T)

    nc = _get_compiled()
    in_maps = [
        {"xt": np.ascontiguousarray(
            xt_full[i * B_LOC:(i + 1) * B_LOC].reshape(B_LOC * T, C_IN, HP,
                                                       NT)),
         "wt": wtd, "bias2": bias2}
        for i in range(N_CORES)
    ]
    res = run_bass_kernel_spmd(nc, in_maps, list(range(N_CORES)), trace=trace)
    outs = []
    for i in range(N_CORES):
        op = res.results[i]["out"].reshape(B_LOC, N_CHUNK, N_RG, 2, 128,
                                           ROWS_PER_IT, NT)
        # out[b, c*128+m, 16d+h, 2j+par] = op[b, c, d, par, m, h, j]
        full = np.transpose(op, (0, 1, 4, 2, 5, 6, 3)).reshape(
            B_LOC, C_OUT, H, W)
        outs.append(full.astype(np.float32))
    return np.concatenate(outs, axis=0), res


def kernel(inp, weight, bias):
    full, _ = _run(inp, weight, bias, trace=False)
    return full


# revision 15
# speedup vs baseline: 2.8773x; 1.0887x over previous
"""3x3 SAME conv (B=32, Cin=128, H=W=64, Cout=256) + bias + relu on 8 trn2 cores.

Strategy: data-parallel over batch (4 images per core, no collectives),
with Winograd F(2,3) along W to cut PE work 1.5x vs direct conv.

The host transforms the input along W (B^T over stride-2 windows of the
zero-padded rows) into 4 t-planes X~t [128cin, 66rows, 32tiles] bf16 per
image, and the weights along kw (G) into W~[chunk, t, kh] [128cin,128cout]
bf16. On device, each iteration (img, chunk, 16-row double-rowgroup) runs
12 matmuls (4 t-planes x 3 kh taps, N=512 moving cols) accumulating 4
PSUM banks Y~t; the Winograd inverse (y_even = Y0+Y1+Y2, y_odd =
Y1-Y2-Y3) runs as 4 fp32 tensor_tensor ops on the vector engine, then
the scalar engine fuses bias+relu+bf16-cast out of SBUF. Even/odd pixel
planes are stored planar to DRAM; the host interleaves and upcasts.

PE work: 32 iters x 12 MM x ~216ns = ~83us; DVE inverse ~84us runs one
iteration behind the PE, psums bufs=8 double-buffers the 4-bank groups.
Startup mirrors the direct-conv baseline: need-ordered sync HWDGE ring
(chunk-0 weights, image-0 row bands, images 1-3), light scalar ring
(bias, chunk-1 weights, stores), and warmup matmuls on a memset tile to
carry the PE clock-gate busy window into the data-ready gate.
"""

from contextlib import ExitStack

import ml_dtypes
import numpy as np

import concourse.bass as bass
import concourse.tile as tile
from concourse import bacc, mybir
from concourse.bass_utils import run_bass_kernel_spmd

N_CORES = 8
B, C_IN, H, W = 32, 128, 64, 64
C_OUT, K = 256, 3
B_LOC = B // N_CORES          # images per core
N_CHUNK = C_OUT // 128        # cout chunks of 128
NT = W // 2                   # Winograd F(2,3) tiles along W
T = 4                         # t-planes
ROWS_PER_IT = 16              # output rows per iteration (N=16*32=512)
N_RG = H // ROWS_PER_IT       # row groups per (image, chunk)
HP = H + 2                    # padded rows

_COMPILED = None


def _build():
    nc = bacc.Bacc("TRN2", target_bir_lowering=False, debug=False,
                   num_devices=N_CORES)

    # X~t planes, indexed [b*T + t] -> [128cin, 66, 32] bf16
    xt = nc.dram_tensor("xt", [B_LOC * T, C_IN, HP, NT], mybir.dt.bfloat16,
                        kind="ExternalInput").ap()
    # W~ chunk-major: wt[c, cin, t*3+kh, m] -> cout m of chunk c
    wt = nc.dram_tensor("wt", [N_CHUNK, C_IN, T * K, 128], mybir.dt.bfloat16,
                        kind="ExternalInput").ap()
    bias2 = nc.dram_tensor("bias2", [128, N_CHUNK], mybir.dt.float32,
                           kind="ExternalInput").ap()
    # planar output: idx = ((b*N_CHUNK + c)*N_RG + d)*2 + parity
    out = nc.dram_tensor("out", [B_LOC * N_CHUNK * N_RG * 2, 128,
                                 ROWS_PER_IT * NT], mybir.dt.bfloat16,
                         kind="ExternalOutput").ap()

    with tile.TileContext(nc) as tc, ExitStack() as ctx:
        consts = ctx.enter_context(tc.tile_pool(name="consts", bufs=1))
        pads = ctx.enter_context(tc.tile_pool(name="pads", bufs=1))
        ys = ctx.enter_context(tc.tile_pool(name="ys", bufs=4))
        outs = ctx.enter_context(tc.tile_pool(name="outs", bufs=6))
        # 4 banks per iteration (4 Y~t planes), double-buffered = all 8 banks
        psums = ctx.enter_context(tc.tile_pool(name="psums", bufs=2,
                                               space="PSUM"))

        w_r = consts.tile([128, N_CHUNK, T * K, 128], mybir.dt.bfloat16,
                          tag="w_r")
        b_sb = consts.tile([128, N_CHUNK], mybir.dt.float32, tag="b_sb")
        nc.scalar.dma_start(out=b_sb[:], in_=bias2[:])

        # Warmup matmuls bridge PE dispatch-ready to data-ready so the HAM
        # clock-gate busy window runs into the real stream. They target the
        # first iteration's PSUM tile (overwritten by its start=True matmul)
        # to keep all 8 banks for the double-buffered Y~t groups.
        warm = consts.tile([128, 512], mybir.dt.bfloat16, tag="warm")
        nc.vector.memset(warm[:], 0.0)

        # X~ SBUF tiles: per image [128, T, 66, 32]
        ximgs = [pads.tile([128, T, HP, NT], mybir.dt.bfloat16,
                           name=f"ximg{i}", tag=f"ximg{i}")
                 for i in range(B_LOC)]

        # Need-ordered data ring: the first iteration's taps are gated per
        # t-plane (weights for t, then image-0 rows 0..17 of t), so the first
        # matmuls can start before later t-planes land; then the rest of
        # image 0 in row bands, then images 1-3 whole.
        nc.scalar.dma_start(out=w_r[:, 1], in_=wt[1])
        for t in range(T):
            nc.sync.dma_start(out=w_r[:, 0, t * K:(t + 1) * K, :],
                              in_=wt[0, :, t * K:(t + 1) * K, :])
            nc.sync.dma_start(out=ximgs[0][:, t, 0:18, :],
                              in_=xt[t, :, 0:18, :])
        bounds = [18, 34, 50, HP]
        for s in range(len(bounds) - 1):
            lo, hi = bounds[s], bounds[s + 1]
            for t in range(T):
                nc.sync.dma_start(out=ximgs[0][:, t, lo:hi, :],
                                  in_=xt[t, :, lo:hi, :])
        for b in range(1, B_LOC):
            for t in range(T):
                nc.sync.dma_start(out=ximgs[b][:, t], in_=xt[b * T + t])

        # Per iteration: 12 matmuls fill 4 PSUM banks Y~0..3; the inverse
        # splits so DVE never reads two PSUM inputs in one op (HW limit) and
        # the SBUF-only combines run in DVE's 2x bf16 mode:
        #   scalar: c1=bf16(Y1), c2=bf16(Y2)     (PSUM -> SBUF stages)
        #   DVE:    va=c1+Y0 (1x), vb=c1-c2 (2x), y_ev=va+c2 (2x),
        #           y_od=vb-Y3 (1x)              (bf16 intermediates)
        #   scalar: o_ev=relu(y_ev+bias), o_od=relu(y_od+bias)
        # The relu+store epilogue of iteration i is EMITTED after iteration
        # i+1's matmuls so no engine FIFO head-blocks on a cross-engine dep.
        pend = None

        def flush_epilogue(last=False):
            nonlocal pend
            if pend is None:
                return
            y_ev, y_od, cc, oidx = pend
            o_ev = outs.tile([128, ROWS_PER_IT * NT], mybir.dt.bfloat16,
                             name="o_ev", tag="o_ev")
            o_od = outs.tile([128, ROWS_PER_IT * NT], mybir.dt.bfloat16,
                             name="o_od", tag="o_od")
            nc.scalar.activation(o_ev[:], y_ev[:],
                                 mybir.ActivationFunctionType.Relu,
                                 bias=b_sb[:, cc:cc + 1], scale=1.0)
            nc.scalar.activation(o_od[:], y_od[:],
                                 mybir.ActivationFunctionType.Relu,
                                 bias=b_sb[:, cc:cc + 1], scale=1.0)
            ring = nc.sync if last else nc.scalar
            ring.dma_start(out=out[oidx], in_=o_ev[:])
            nc.scalar.dma_start(out=out[oidx + 1], in_=o_od[:])
            pend = None

        for b in range(B_LOC):
            ximg = ximgs[b]
            for c in range(N_CHUNK):
                for d in range(N_RG):
                    y0r = d * ROWS_PER_IT
                    yt = [psums.tile([128, ROWS_PER_IT * NT],
                                     mybir.dt.float32, name=f"yt{t}",
                                     tag=f"yt{t}")
                          for t in range(T)]
                    if b == 0 and c == 0 and d == 0:
                        for i in range(13):
                            nc.tensor.matmul(yt[0][:, 0:256], warm[:, 0:128],
                                             warm[:, 0:256], start=True,
                                             stop=True)
                    for t in range(T):
                        for kh in range(K):
                            rhs = ximg[:, t, y0r + kh:y0r + kh + ROWS_PER_IT,
                                       :]
                            nc.tensor.matmul(yt[t][:],
                                             w_r[:, c, t * K + kh, :],
                                             rhs,
                                             start=(kh == 0),
                                             stop=(kh == K - 1))
                    it = (b * N_CHUNK + c) * N_RG + d
                    c1 = ys.tile([128, ROWS_PER_IT * NT], mybir.dt.bfloat16,
                                 tag="c1")
                    c2 = ys.tile([128, ROWS_PER_IT * NT], mybir.dt.bfloat16,
                                 tag="c2")
                    va = ys.tile([128, ROWS_PER_IT * NT], mybir.dt.bfloat16,
                                 tag="va")
                    vb = ys.tile([128, ROWS_PER_IT * NT], mybir.dt.bfloat16,
                                 tag="vb")
                    y_ev = ys.tile([128, ROWS_PER_IT * NT], mybir.dt.bfloat16,
                                   tag="y_ev")
                    y_od = ys.tile([128, ROWS_PER_IT * NT], mybir.dt.bfloat16,
                                   tag="y_od")
                    # Stage Y1/Y2 before the previous iteration's relus in the
                    # scalar queue (their DVE consumers are next); alternate
                    # the c2 copy between scalar and DVE to balance both.
                    nc.scalar.copy(c1[:], yt[1][:])
                    if it % 2 == 0:
                        nc.scalar.copy(c2[:], yt[2][:])
                    else:
                        nc.vector.tensor_copy(c2[:], yt[2][:])
                    flush_epilogue()
                    nc.vector.tensor_tensor(va[:], c1[:], yt[0][:],
                                            mybir.AluOpType.add)
                    nc.vector.tensor_tensor(vb[:], c1[:], c2[:],
                                            mybir.AluOpType.subtract)
                    nc.vector.tensor_tensor(y_ev[:], va[:], c2[:],
                                            mybir.AluOpType.add)
                    nc.vector.tensor_tensor(y_od[:], vb[:], yt[3][:],
                                            mybir.AluOpType.subtract)
                    pend = (y_ev, y_od, c, it * 2)
        flush_epilogue(last=True)

    nc.compile()
    return nc


def _get_compiled():
    global _COMPILED
    if _COMPILED is None:
        _COMPILED = _build()
    return _COMPILED


# F(2,3) transform matrices (host side, fp32 exact)
_BT = np.array([[1, 0, -1, 0], [0, 1, 1, 0], [0, -1, 1, 0], [0, 1, 0, -1]],
               dtype=np.float32)
_G = np.array([[1, 0, 0], [.5, .5, .5], [.5, -.5, .5], [0, 0, 1]],
              dtype=np.float32)


def _run(inp, weight, bias, trace=False):
    inp = np.asarray(inp, dtype=np.float32)
    weight = np.asarray(weight, dtype=np.float32)
    bias = np.asarray(bias, dtype=np.float32)

    # Host: zero-pad, Winograd-transform along W, cast bf16.
    x = np.zeros((B, C_IN, HP, W + 2), dtype=np.float32)
    x[:, :, 1:H + 1, 1:W + 1] = inp
    idx = 2 * np.arange(NT)[:, None] + np.arange(T)[None, :]   # [NT, T]
    xg = x[:, :, :, idx]                                        # [B,C,HP,NT,T]
    xt_full = np.einsum('tk,bchjk->btchj', _BT, xg)             # [B,T,C,HP,NT]
    xt_full = xt_full.astype(ml_dtypes.bfloat16)

    # weight [C_OUT, C_IN*9] -> W~[t,kh,cin,cout] -> [chunk, cin, t*3+kh, m]
    w4 = weight.reshape(C_OUT, C_IN, K, K)
    wtf = np.einsum('tk,ochk->thco', _G, w4)                    # [T,K,C_IN,C_OUT]
    wtd = np.ascontiguousarray(
        wtf.reshape(T * K, C_IN, N_CHUNK, 128).transpose(2, 1, 0, 3)
    ).astype(ml_dtypes.bfloat16)
    bias2 = np.ascontiguousarray(bias.reshape(N_CHUNK, 128).T)

    nc = _get_compiled()
    in_maps = [
        {"xt": np.ascontiguousarray(
            xt_full[i * B_LOC:(i + 1) * B_LOC].reshape(B_LOC * T, C_IN, HP,
                                                       NT)),
         "wt": wtd, "bias2": bias2}
        for i in range(N_CORES)
    ]
    res = run_bass_kernel_spmd(nc, in_maps, list(range(N_CORES)), trace=trace)
    outs = []
    for i in range(N_CORES):
        op = res.results[i]["out"].reshape(B_LOC, N_CHUNK, N_RG, 2, 128,
                                           ROWS_PER_IT, NT)
        # out[b, c*128+m, 16d+h, 2j+par] = op[b, c, d, par, m, h, j]
        full = np.transpose(op, (0, 1, 4, 2, 5, 6, 3)).reshape(
            B_LOC, C_OUT, H, W)
        outs.append(full.astype(np.float32))
    return np.concatenate(outs, axis=0), res


def kernel(inp, weight, bias):
    full, _ = _run(inp, weight, bias, trace=False)
    return full


# revision 19
# speedup vs baseline: 3.2807x; 1.1402x over previous
"""3x3 SAME conv (B=32, Cin=128, H=W=64, Cout=256) + bias + relu on 8 trn2 cores.

Strategy: data-parallel over batch (4 images per core, no collectives),
with Winograd F(2,3) along W to cut PE work 1.5x vs direct conv.

The host transforms the input along W (B^T over stride-2 windows of the
zero-padded rows) into 4 t-planes X~t [128cin, 66rows, 32tiles] bf16 per
image, and the weights along kw (G) into W~[chunk, t, kh] [128cin,128cout]
bf16. On device, each iteration (img, chunk, 16-row double-rowgroup) runs
12 matmuls (4 t-planes x 3 kh taps, N=512 moving cols) accumulating 4
PSUM banks Y~t; the Winograd inverse (y_even = Y0+Y1+Y2, y_odd =
Y1-Y2-Y3) runs as 4 fp32 tensor_tensor ops on the vector engine, then
the scalar engine fuses bias+relu+bf16-cast out of SBUF. Even/odd pixel
planes are stored planar to DRAM; the host interleaves and upcasts.

PE work: 32 iters x 12 MM x ~216ns = ~83us; DVE inverse ~84us runs one
iteration behind the PE, psums bufs=8 double-buffers the 4-bank groups.
Startup mirrors the direct-conv baseline: need-ordered sync HWDGE ring
(chunk-0 weights, image-0 row bands, images 1-3), light scalar ring
(bias, chunk-1 weights, stores), and warmup matmuls on a memset tile to
carry the PE clock-gate busy window into the data-ready gate.
"""

from contextlib import ExitStack

import ml_dtypes
import numpy as np

import concourse.bass as bass
import concourse.tile as tile
from concourse import bacc, mybir
from concourse.bass_utils import run_bass_kernel_spmd

N_CORES = 8
B, C_IN, H, W = 32, 128, 64, 64
C_OUT, K = 256, 3
B_LOC = B // N_CORES          # images per core
N_CHUNK = C_OUT // 128        # cout chunks of 128
NT = W // 2                   # Winograd F(2,3) tiles along W
T = 4                         # t-planes
ROWS_PER_IT = 16              # output rows per iteration (N=16*32=512)
N_RG = H // ROWS_PER_IT       # row groups per (image, chunk)
HP = H + 2                    # padded rows

_COMPILED = None


def _build():
    nc = bacc.Bacc("TRN2", target_bir_lowering=False, debug=False,
                   num_devices=N_CORES)

    # X~t planes, indexed [b*T + t] -> [128cin, 66, 32] bf16
    xt = nc.dram_tensor("xt", [B_LOC * T, C_IN, HP, NT], mybir.dt.bfloat16,
                        kind="ExternalInput").ap()
    # W~ chunk-major: wt[c, cin, t*3+kh, m] -> cout m of chunk c
    wt = nc.dram_tensor("wt", [N_CHUNK, C_IN, T * K, 128], mybir.dt.bfloat16,
                        kind="ExternalInput").ap()
    bias2 = nc.dram_tensor("bias2", [128, N_CHUNK], mybir.dt.float32,
                           kind="ExternalInput").ap()
    # planar output per iteration: [it, cout_m, parity, h*w~]
    out = nc.dram_tensor("out", [B_LOC * N_CHUNK * N_RG, 128,
                                 2 * ROWS_PER_IT * NT], mybir.dt.bfloat16,
                         kind="ExternalOutput").ap()

    with tile.TileContext(nc) as tc, ExitStack() as ctx:
        consts = ctx.enter_context(tc.tile_pool(name="consts", bufs=1))
        pads = ctx.enter_context(tc.tile_pool(name="pads", bufs=1))
        ys = ctx.enter_context(tc.tile_pool(name="ys", bufs=4))
        outs = ctx.enter_context(tc.tile_pool(name="outs", bufs=6))
        # 4 banks per iteration (4 Y~t planes), double-buffered = all 8 banks
        psums = ctx.enter_context(tc.tile_pool(name="psums", bufs=2,
                                               space="PSUM"))

        w_r = consts.tile([128, N_CHUNK, T * K, 128], mybir.dt.bfloat16,
                          tag="w_r")
        b_sb = consts.tile([128, N_CHUNK], mybir.dt.float32, tag="b_sb")
        nc.scalar.dma_start(out=b_sb[:], in_=bias2[:])

        # Warmup matmuls bridge PE dispatch-ready to data-ready so the HAM
        # clock-gate busy window runs into the real stream. They target the
        # first iteration's PSUM tile (overwritten by its start=True matmul)
        # to keep all 8 banks for the double-buffered Y~t groups.
        warm = consts.tile([128, 512], mybir.dt.bfloat16, tag="warm")
        nc.vector.memset(warm[:], 0.0)

        # X~ SBUF tiles: per image [128, T, 66, 32]
        ximgs = [pads.tile([128, T, HP, NT], mybir.dt.bfloat16,
                           name=f"ximg{i}", tag=f"ximg{i}")
                 for i in range(B_LOC)]

        # Need-ordered data ring: the first iteration's taps are gated per
        # t-plane (weights for t, then image-0 rows 0..17 of t), so the first
        # matmuls can start before later t-planes land; then the rest of
        # image 0 in row bands, then images 1-3 whole.
        nc.scalar.dma_start(out=w_r[:, 1], in_=wt[1])
        for t in range(T):
            nc.sync.dma_start(out=w_r[:, 0, t * K:(t + 1) * K, :],
                              in_=wt[0, :, t * K:(t + 1) * K, :])
            nc.sync.dma_start(out=ximgs[0][:, t, 0:18, :],
                              in_=xt[t, :, 0:18, :])
        bounds = [18, 34, 50, HP]
        for s in range(len(bounds) - 1):
            lo, hi = bounds[s], bounds[s + 1]
            for t in range(T):
                nc.sync.dma_start(out=ximgs[0][:, t, lo:hi, :],
                                  in_=xt[t, :, lo:hi, :])
        for b in range(1, B_LOC):
            for t in range(T):
                nc.sync.dma_start(out=ximgs[b][:, t], in_=xt[b * T + t])

        # Per iteration: 12 matmuls fill 4 PSUM banks (Y~1/Y~2 share one
        # 2-bank tile); the inverse splits so DVE never reads two PSUM
        # inputs in one op (HW limit), with per-op init overheads amortized
        # by batching the PSUM stage, the relu, and the store at FD=1024:
        #   scalar: c12 = bf16(Y1|Y2)            (one 2-bank PSUM copy)
        #   DVE:    va=c1+Y0 (1x), vb=c1-c2 (2x), y_ev=va+c2 (2x),
        #           y_od=vb-Y3 (1x)              (bf16, into one pair tile)
        #   scalar: o_pair = relu(y_pair+bias); one store per iteration.
        # The relu+store epilogue of iteration i is EMITTED after iteration
        # i+1's matmuls so no engine FIFO head-blocks on a cross-engine dep.
        pend = None

        def flush_epilogue(last=False):
            nonlocal pend
            if pend is None:
                return
            y_pair, cc, oidx = pend
            o_pair = outs.tile([128, 2, ROWS_PER_IT * NT], mybir.dt.bfloat16,
                               name="o_pair", tag="o_pair")
            nc.scalar.activation(o_pair[:], y_pair[:],
                                 mybir.ActivationFunctionType.Relu,
                                 bias=b_sb[:, cc:cc + 1], scale=1.0)
            ring = nc.sync if last else nc.scalar
            ring.dma_start(out=out[oidx], in_=o_pair[:])
            pend = None

        for b in range(B_LOC):
            ximg = ximgs[b]
            for c in range(N_CHUNK):
                for d in range(N_RG):
                    y0r = d * ROWS_PER_IT
                    yt0 = psums.tile([128, ROWS_PER_IT * NT],
                                     mybir.dt.float32, tag="yt0")
                    y12 = psums.tile([128, 2, ROWS_PER_IT * NT],
                                     mybir.dt.float32, tag="y12")
                    yt3 = psums.tile([128, ROWS_PER_IT * NT],
                                     mybir.dt.float32, tag="yt3")
                    mm_dst = [yt0[:], y12[:, 0], y12[:, 1], yt3[:]]
                    if b == 0 and c == 0 and d == 0:
                        for i in range(13):
                            nc.tensor.matmul(yt0[:, 0:256], warm[:, 0:128],
                                             warm[:, 0:256], start=True,
                                             stop=True)
                    for t in range(T):
                        for kh in range(K):
                            rhs = ximg[:, t, y0r + kh:y0r + kh + ROWS_PER_IT,
                                       :]
                            nc.tensor.matmul(mm_dst[t],
                                             w_r[:, c, t * K + kh, :],
                                             rhs,
                                             start=(kh == 0),
                                             stop=(kh == K - 1))
                    it = (b * N_CHUNK + c) * N_RG + d
                    c12 = ys.tile([128, 2, ROWS_PER_IT * NT],
                                  mybir.dt.bfloat16, tag="c12")
                    va = ys.tile([128, ROWS_PER_IT * NT], mybir.dt.bfloat16,
                                 tag="va")
                    vb = ys.tile([128, ROWS_PER_IT * NT], mybir.dt.bfloat16,
                                 tag="vb")
                    y_pair = ys.tile([128, 2, ROWS_PER_IT * NT],
                                     mybir.dt.bfloat16, tag="y_pair")
                    # Stage Y1|Y2 in one 2-bank PSUM read, emitted before the
                    # previous iteration's relu in the scalar queue (its DVE
                    # consumers come next).
                    nc.scalar.copy(c12[:], y12[:])
                    flush_epilogue()
                    nc.vector.tensor_tensor(va[:], c12[:, 0], yt0[:],
                                            mybir.AluOpType.add)
                    nc.vector.tensor_tensor(vb[:], c12[:, 0], c12[:, 1],
                                            mybir.AluOpType.subtract)
                    nc.vector.tensor_tensor(y_pair[:, 0], va[:], c12[:, 1],
                                            mybir.AluOpType.add)
                    nc.vector.tensor_tensor(y_pair[:, 1], vb[:], yt3[:],
                                            mybir.AluOpType.subtract)
                    pend = (y_pair, c, it)
        flush_epilogue(last=True)

    nc.compile()
    return nc


def _get_compiled():
    global _COMPILED
    if _COMPILED is None:
        _COMPILED = _build()
    return _COMPILED


# F(2,3) transform matrices (host side, fp32 exact)
_BT = np.array([[1, 0, -1, 0], [0, 1, 1, 0], [0, -1, 1, 0], [0, 1, 0, -1]],
               dtype=np.float32)
_G = np.array([[1, 0, 0], [.5, .5, .5], [.5, -.5, .5], [0, 0, 1]],
              dtype=np.float32)


def _run(inp, weight, bias, trace=False):
    inp = np.asarray(inp, dtype=np.float32)
    weight = np.asarray(weight, dtype=np.float32)
    bias = np.asarray(bias, dtype=np.float32)

    # Host: zero-pad, Winograd-transform along W, cast bf16.
    x = np.zeros((B, C_IN, HP, W + 2), dtype=np.float32)
    x[:, :, 1:H + 1, 1:W + 1] = inp
    idx = 2 * np.arange(NT)[:, None] + np.arange(T)[None, :]   # [NT, T]
    xg = x[:, :, :, idx]                                        # [B,C,HP,NT,T]
    xt_full = np.einsum('tk,bchjk->btchj', _BT, xg)             # [B,T,C,HP,NT]
    xt_full = xt_full.astype(ml_dtypes.bfloat16)

    # weight [C_OUT, C_IN*9] -> W~[t,kh,cin,cout] -> [chunk, cin, t*3+kh, m]
    w4 = weight.reshape(C_OUT, C_IN, K, K)
    wtf = np.einsum('tk,ochk->thco', _G, w4)                    # [T,K,C_IN,C_OUT]
    wtd = np.ascontiguousarray(
        wtf.reshape(T * K, C_IN, N_CHUNK, 128).transpose(2, 1, 0, 3)
    ).astype(ml_dtypes.bfloat16)
    bias2 = np.ascontiguousarray(bias.reshape(N_CHUNK, 128).T)

    nc = _get_compiled()
    in_maps = [
        {"xt": np.ascontiguousarray(
            xt_full[i * B_LOC:(i + 1) * B_LOC].reshape(B_LOC * T, C_IN, HP,
                                                       NT)),
         "wt": wtd, "bias2": bias2}
        for i in range(N_CORES)
    ]
    res = run_bass_kernel_spmd(nc, in_maps, list(range(N_CORES)), trace=trace)
    outs = []
    for i in range(N_CORES):
        op = res.results[i]["out"].reshape(B_LOC, N_CHUNK, N_RG, 128, 2,
                                           ROWS_PER_IT, NT)
        # out[b, c*128+m, 16d+h, 2j+par] = op[b, c, d, m, par, h, j]
        full = np.transpose(op, (0, 1, 3, 2, 5, 6, 4)).reshape(
            B_LOC, C_OUT, H, W)
        outs.append(full.astype(np.float32))
    return np.concatenate(outs, axis=0), res


def kernel(inp, weight, bias):
    full, _ = _run(inp, weight, bias, trace=False)
    return full
